# revision 10
# baseline (speedup 1.0000x reference)
"""Trainium2 Bass kernel for nn_InterpretableAttention (B=8, N=4096, DIM=1024).

Math: the reference returns softmax(q @ k^T, axis=-1)[:, 0, :] -- only row 0
of the attention matrix. Per batch b:
    q0       = Wq @ x[b,0] + bq                     [DIM]
    v        = Wk^T @ q0 = M @ x[b,0] + c           [DIM]
               with M = Wk^T Wq, c = Wk^T bq  (weight-only fold, host-side)
    scores_m = x[b,m] . v   (+ q0.bk, constant -> cancels in softmax)
    out[b]   = softmax(scores)                      [N]
bk never affects the output. The N x N score matrix and the full q/k
projections are never materialized.

Sharding: data-parallel over batch, one batch per NeuronCore (B == 8 cores).
M (fp16, 2 MB) is replicated; each core computes its own v on-device via 64
[128,128]x[128,1] matmuls, then streams its batch's x (fp16, host-cast,
transposed to [DIM, N]) through 64 accumulating [128,1]x[128,512] matmuls.
fp16 halves both HBM traffic and PE cycles vs fp32 (4 cyc/row -> 1).

Per-core device pipeline:
  0) ~40 tiny zero matmuls at t=0 warm the PE (HAM ramps 1.2->2.4 GHz).
  A) v = M16 @ x0 + c: 64 accumulating matmuls (M^T blocks stationary),
     DVE bias-add + fp16 cast.
  B) 8 k-slice DMAs of x^T ([128,4096] fp16, 1 MB each, 8KB/partition
     contiguous) alternating sync/scalar HWDGE queues; per slice 8
     matmuls accumulate into 8 PSUM banks ([1,512] each).
  C) online softmax: per m-tile local max (DVE) + exp/accumulate (ACT),
     then a tiny combine (global max, sum of scaled tile sums) and a
     per-tile rescale split across DVE/ACT; out DMA in two halves.
"""

import os
from contextlib import ExitStack

import numpy as np

import concourse.bass as bass  # noqa: F401
import concourse.tile as tile
from concourse import bacc, mybir
from concourse.bass_utils import run_bass_kernel_spmd

B, N, DIM = 8, 4096, 1024
P = 128          # partitions
KC = DIM // P    # 8 chunks along d
MT = 512         # m-tile (matmul moving free dim / PSUM bank)
NMT = N // MT    # 8 m-tiles
F32 = mybir.dt.float32
F16 = mybir.dt.float16
NWARM = int(os.environ.get("KERNEL_NWARM", "40"))
BARRIER = os.environ.get("KERNEL_BARRIER", "0") == "1"

_program_cache = {}


def _build_program(reps: int = 1):
    key = (reps, BARRIER)
    if key in _program_cache:
        return _program_cache[key]

    nc = bacc.Bacc(
        "TRN2",
        target_bir_lowering=False,
        debug=False,
        enable_asserts=False,
        num_devices=B,
    )
    # Host-prepared, per-core DRAM inputs (all partition-contiguous):
    #   xt [DIM, N] f16: x[b]^T
    #   mt [P, KC*KC*P] f16: M^T blocks, mt[p, k, j, e] = M[j*128+e, k*128+p]
    #   x0 [P, KC] f16: x0[p, c] = x[b, 0, c*128+p]
    #   ct [P, KC] f32: c[p, c'] = (Wk^T bq)[c'*128+p]
    xt = nc.dram_tensor("xt", [DIM, N], F16, kind="ExternalInput").ap()
    mt = nc.dram_tensor("mt", [P, KC * KC * P], F16, kind="ExternalInput").ap()
    x0 = nc.dram_tensor("x0", [P, KC], F16, kind="ExternalInput").ap()
    ct = nc.dram_tensor("ct", [P, KC], F32, kind="ExternalInput").ap()
    out = nc.dram_tensor("out", [1, N], F32, kind="ExternalOutput").ap()

    with tile.TileContext(nc) as tc, ExitStack() as ctx:
        singles = ctx.enter_context(tc.tile_pool(name="singles", bufs=1))
        wpool = ctx.enter_context(tc.tile_pool(name="wpool", bufs=2))
        xpool = ctx.enter_context(tc.tile_pool(name="xpool", bufs=KC))
        pspool = ctx.enter_context(tc.tile_pool(name="pspool", bufs=8, space="PSUM"))

        # ---- PE warmup: keep the PE busy from t=0 so HAM ramps to 2.4 GHz
        zt = singles.tile([P, 64], F16)
        nc.gpsimd.memset(zt, 0.0)
        wps = pspool.tile([64, 64], F32, name="pst")
        for _ in range(NWARM):
            nc.tensor.matmul(wps, zt, zt, start=True, stop=True)

        for _ in range(reps):
            # ---------------- Phase A: v = M @ x0 + c ----------------
            x0s = wpool.tile([P, KC], F16)
            nc.sync.dma_start(x0s, x0)
            cs = wpool.tile([P, KC], F32)
            nc.sync.dma_start(cs, ct)
            mts = wpool.tile([P, KC, KC, P], F16)
            half = KC * KC * P // 2
            mtr = mt.rearrange("p (k j e) -> p k j e", k=KC, j=KC)
            nc.sync.dma_start(mts[:, : KC // 2], mtr[:, : KC // 2])
            nc.scalar.dma_start(mts[:, KC // 2 :], mtr[:, KC // 2 :])

            vps = pspool.tile([P, KC], F32, name="pst")
            for j in range(KC):
                for k in range(KC):
                    nc.tensor.matmul(
                        vps[:, j : j + 1],
                        mts[:, k, j, :],
                        x0s[:, k : k + 1],
                        start=(k == 0),
                        stop=(k == KC - 1),
                    )
            vs16 = wpool.tile([P, KC], F16)
            nc.vector.tensor_add(vs16, vps, cs)

            # ---------------- Phase B: scores[m] = x[m] . v ----------------
            ps = []
            for t in range(NMT):
                pst = pspool.tile([1, MT], F32, name="pst")
                ps.append(pst)
            esb = singles.tile([1, N], F32)
            osb = singles.tile([1, N], F32)
            nmax = singles.tile([1, NMT], F32)
            ssum = singles.tile([1, NMT], F32)

            for k in range(KC):
                xk = xpool.tile([P, N], F16, name="xk")
                if k < KC - 1:
                    eng = nc.sync if k % 2 == 0 else nc.scalar
                    eng.dma_start(xk, xt[k * P : (k + 1) * P, :])
                    for t in range(NMT):
                        nc.tensor.matmul(
                            ps[t],
                            vs16[:, k : k + 1],
                            xk[:, t * MT : (t + 1) * MT],
                            start=(k == 0),
                            stop=False,
                        )
                else:
                    # last k-slice in 8 m-chunks: its matmuls + per-tile
                    # softmax pipeline with the DMA tail (subtile deps)
                    for t in range(NMT):
                        sl = slice(t * MT, (t + 1) * MT)
                        eng = nc.sync if t % 2 == 0 else nc.scalar
                        eng.dma_start(xk[:, sl], xt[k * P : (k + 1) * P, sl])
                        nc.tensor.matmul(
                            ps[t], vs16[:, k : k + 1], xk[:, sl],
                            start=False, stop=True,
                        )
                        # ---- Phase C (online): local max, exp, local sum
                        nc.vector.tensor_reduce(
                            nmax[:, t : t + 1],
                            ps[t],
                            axis=mybir.AxisListType.X,
                            op=mybir.AluOpType.max,
                            negate=True,
                        )
                        nc.scalar.activation(
                            esb[:, sl],
                            ps[t],
                            mybir.ActivationFunctionType.Exp,
                            bias=nmax[:, t : t + 1],
                            scale=1.0,
                            accum_out=ssum[:, t : t + 1],
                        )

            # ---- combine: g = max m_t; S = sum_t sigma_t*exp(m_t-g)
            gneg = singles.tile([1, 1], F32)
            nc.vector.tensor_reduce(
                gneg, nmax, axis=mybir.AxisListType.X, op=mybir.AluOpType.min
            )
            phi = singles.tile([1, NMT], F32)
            nc.scalar.activation(
                phi, nmax, mybir.ActivationFunctionType.Exp, bias=gneg, scale=-1.0
            )
            w8 = singles.tile([1, NMT], F32)
            nc.vector.tensor_mul(w8, phi, ssum)
            S = singles.tile([1, 1], F32)
            nc.vector.tensor_reduce(
                S, w8, axis=mybir.AxisListType.X, op=mybir.AluOpType.add
            )
            rinv = singles.tile([1, 1], F32)
            nc.vector.reciprocal(rinv, S)
            alpha = singles.tile([1, NMT], F32)
            nc.vector.tensor_scalar_mul(alpha, phi, rinv)

            # ---- rescale each tile by alpha_t (DVE/ACT split), out DMA
            # per tile-pair so the out DMAs overlap the remaining scales
            for t in range(NMT):
                sl = slice(t * MT, (t + 1) * MT)
                if t % 2 == 0:
                    nc.vector.tensor_scalar_mul(osb[:, sl], esb[:, sl], alpha[:, t : t + 1])
                else:
                    nc.scalar.mul(osb[:, sl], esb[:, sl], alpha[:, t : t + 1])
                    osl = slice((t - 1) * MT, (t + 1) * MT)
                    eng = nc.sync if (t // 2) % 2 == 0 else nc.scalar
                    eng.dma_start(out[:, osl], osb[:, osl])
            if BARRIER:
                tc.strict_bb_all_engine_barrier()

    nc.compile()
    _program_cache[key] = nc
    return nc


def _make_in_maps(x, Wq, bq, Wk):
    x = np.asarray(x, dtype=np.float32)
    wq = np.asarray(Wq, np.float32)
    wk = np.asarray(Wk, np.float32)
    bq = np.asarray(bq, np.float32)

    M = (wk.T @ wq).astype(np.float32)  # [D, D]
    c = (wk.T @ bq).astype(np.float32)  # [D]
    # mt[p, k, j, e] = M[j*128+e, k*128+p]
    mt_h = np.ascontiguousarray(
        M.reshape(KC, P, KC, P).transpose(3, 2, 0, 1).reshape(P, KC * KC * P)
    ).astype(np.float16)
    ct_h = np.ascontiguousarray(c.reshape(KC, P).T)  # [P, KC] f32
    x16 = x.astype(np.float16)

    in_maps = []
    for b in range(B):
        in_maps.append(
            {
                "xt": np.ascontiguousarray(x16[b].T),  # [DIM, N] f16
                "mt": mt_h,
                "x0": np.ascontiguousarray(x16[b, 0].reshape(KC, P).T),  # [P,KC]
                "ct": ct_h,
            }
        )
    return in_maps


def kernel(x, Wq, bq, Wk, bk):
    nc = _build_program()
    in_maps = _make_in_maps(x, Wq, bq, Wk)
    res = run_bass_kernel_spmd(nc, in_maps, core_ids=list(range(B)))
    outs = [np.asarray(res.results[b]["out"]).reshape(N) for b in range(B)]
    return np.stack(outs, axis=0).astype(np.float32)
